# revision 19
# baseline (speedup 1.0000x reference)
"""Trainium2 kernel for nn_ContrastiveLoss (N=4096, D=1024), SPMD over 8 NeuronCores.

Strategy (row-sharded similarity matrix):
  - Host: l2-normalize back_VF/back_AF (f64 -> bf16), pre-transpose into
    matmul-friendly blocked layouts, compute the diagonal sims (O(N*D)).
  - Each core c: computes its [512, 4096] slab of E = exp(Vn @ An^T):
      * TensorE: 256 accumulating matmuls (K=1024 in 8 chunks, N in 8x512 cols)
      * ScalarE: exp(PSUM) -> bf16 SBUF tile, fused row-sum via accum_out
      * VectorE: partition-folded column-sum accumulation + pre_cos reductions
      * TensorE (ones-matmul): fold 128 partitions of the column sums
    Outputs per core: rowsum chunks [128, 32], partial colsum [1, 4096],
    pre-feature dot/normsq reductions [128, 12].
  - Host: O(N) final assembly (log/ratio/sums) in f64.
"""

import os
import sys

import numpy as np

for _p in ("/opt/trn_rl_repo",):
    if _p not in sys.path and os.path.isdir(_p):
        sys.path.insert(0, _p)

N = 4096
D = 1024
NCORES = 8
ROWS = N // NCORES       # 512 rows per core
MCH = ROWS // 128        # 4 row chunks per core
KCH = D // 128           # 8 contraction chunks
NB = 512                 # matmul moving free dim
NCH = N // NB            # 8 column blocks

MARGIN = 0.2
BALANCE = 0.5
BIAS = 1.0
EPS = 1e-18

_CACHE = {}
LAST_RESULT = None  # BassKernelResults of the most recent run (for test harness)


def _build_nc():
    import concourse.bass as bass  # noqa: F401
    import concourse.bacc as bacc
    import concourse.tile as tile
    from concourse import mybir
    from contextlib import ExitStack

    BF16 = mybir.dt.bfloat16
    F32 = mybir.dt.float32
    Exp = mybir.ActivationFunctionType.Exp
    Square = mybir.ActivationFunctionType.Square
    mult = mybir.AluOpType.mult

    NP2 = NCH // 2  # column-block pairs; each ACT/exp covers 1024 cols

    nc = bacc.Bacc("TRN2", debug=False, num_devices=NCORES)

    # DRAM I/O (per core). Layouts chosen so every DMA is contiguous.
    # vnT[p, k*ROWS + m] = Vn_slab[m, k*128 + p]
    vnT_d = nc.dram_tensor("vnT", [128, KCH * ROWS], BF16, kind="ExternalInput")
    # anT[n, p, k*NB + c] = An[n*NB + c, k*128 + p]
    anT_d = nc.dram_tensor("anT", [NCH, 128, KCH * NB], BF16, kind="ExternalInput")
    # preX[m, p, :] = pre_X_slab[m*128 + p, :]
    preV_d = nc.dram_tensor("preV", [MCH, 128, D], BF16, kind="ExternalInput")
    preA_d = nc.dram_tensor("preA", [MCH, 128, D], BF16, kind="ExternalInput")

    # rowsum[p, np2*MCH + m] = sum over cols [np2*1024,(np2+1)*1024) of
    #   E_slab[m*128 + p, :]
    rowsum_d = nc.dram_tensor("rowsum", [128, NCH // 2 * MCH], F32, kind="ExternalOutput")
    # colsum[0, j] = sum over this core's 512 rows of E[:, j]
    colsum_d = nc.dram_tensor("colsum", [1, N], F32, kind="ExternalOutput")
    # pre3[p, 3*m + {0,1,2}] = dot/nv/na of slab row m*128+p
    pre3_d = nc.dram_tensor("pre3", [128, 3 * MCH], F32, kind="ExternalOutput")

    with tile.TileContext(nc) as tc:
        with ExitStack() as ctx:
            singles = ctx.enter_context(tc.tile_pool(name="singles", bufs=1))

            # DMA issue is ~600ns per dma_start on the issuing engine, so
            # spread issues over idle engines (round robin).
            dma_engines = [nc.sync, nc.gpsimd]
            _rr = [0]

            def dma(out_ap, in_ap):
                eng = dma_engines[_rr[0] % len(dma_engines)]
                _rr[0] += 1
                eng.dma_start(out_ap, in_ap)

            vn_sb = singles.tile([128, KCH * ROWS], BF16, tag="vn")
            half = KCH * ROWS // 2
            dma(vn_sb[:, :half], vnT_d.ap()[:, :half])
            dma(vn_sb[:, half:], vnT_d.ap()[:, half:])

            an_sb = []
            for n in range(NCH):
                t = singles.tile([128, KCH * NB], BF16, tag=f"an{n}")
                # the first blocks gate the matmul stream: split them wider
                # so more DMA queues work on them in parallel
                nsub = 8 if n < 2 else 4
                q = KCH * NB // nsub
                for j in range(nsub):
                    dma(t[:, j * q : (j + 1) * q], anT_d.ap()[n][:, j * q : (j + 1) * q])
                an_sb.append(t)

            efold = singles.tile([128, N], F32, tag="efold")
            efold16 = singles.tile([128, N], BF16, tag="efold16")
            rs = singles.tile([128, NP2 * MCH], F32, tag="rs")
            pre3 = singles.tile([128, 3 * MCH], F32, tag="pre3")
            ones_b = singles.tile([128, 1], BF16, tag="ones_b")
            nc.vector.memset(ones_b[:], 1.0)
            colsb = singles.tile([1, N], F32, tag="colsb")
            dummy = singles.tile([128, NB], BF16, tag="dummy")
            nc.gpsimd.memset(dummy[:], 0.0)

            psum = ctx.enter_context(tc.tile_pool(name="mm_psum", bufs=3, space="PSUM"))
            foldp = ctx.enter_context(tc.tile_pool(name="fold_psum", bufs=2, space="PSUM"))
            epool = ctx.enter_context(tc.tile_pool(name="etile", bufs=3))

            # HAM warmup: keep TensorE busy during the initial DMA wait so the
            # clock gate is at 8/8 when the real matmul stream starts.
            wps = foldp.tile([128, NB], mybir.dt.float32, tag="fold")
            for i in range(8):
                nc.tensor.matmul(
                    wps[0:1, :], ones_b[:], dummy[:], start=(i == 0), stop=(i == 7)
                )

            # Main similarity slab. Column-pair outer (np2), row-chunk inner:
            # each group accumulates 16 matmuls into a [128, 1024] PSUM tile
            # (2 banks), then one wide exp (fused row-sum) drains it.
            # Column sums accumulate in f32 (m=0 written by exp directly,
            # m=3 add emits bf16) and are partition-folded by bf16
            # ones-matmuls, software-pipelined one pair behind the stream.
            def fold(np2):
                for j in range(2):
                    nn = 2 * np2 + j
                    fps = foldp.tile([128, NB], mybir.dt.float32, tag="fold")
                    nc.tensor.matmul(
                        fps[0:1, :],
                        ones_b[:],
                        efold16[:, nn * NB : (nn + 1) * NB],
                        start=True,
                        stop=True,
                    )
                    nc.scalar.copy(colsb[:, nn * NB : (nn + 1) * NB], fps[0:1, :])

            for np2 in range(NP2):
                nlo, nhi = 2 * np2, 2 * np2 + 1
                for m in range(MCH):
                    ps = psum.tile([128, 2 * NB], mybir.dt.float32)
                    for k in range(KCH):
                        w = vn_sb[:, k * ROWS + m * 128 : k * ROWS + (m + 1) * 128]
                        nc.tensor.matmul(
                            ps[:, 0:NB],
                            w,
                            an_sb[nlo][:, k * NB : (k + 1) * NB],
                            start=(k == 0),
                            stop=(k == KCH - 1),
                        )
                        nc.tensor.matmul(
                            ps[:, NB : 2 * NB],
                            w,
                            an_sb[nhi][:, k * NB : (k + 1) * NB],
                            start=(k == 0),
                            stop=(k == KCH - 1),
                        )
                    if m == 0 and np2 > 0:
                        # previous pair's partition fold, emitted here so the
                        # PE never waits on the exp/add chain
                        fold(np2 - 1)
                    col = np2 * MCH + m
                    sl = slice(np2 * 2 * NB, (np2 + 1) * 2 * NB)
                    if m == 0:
                        nc.scalar.activation(
                            efold[:, sl], ps[:], Exp, accum_out=rs[:, col : col + 1]
                        )
                    else:
                        et = epool.tile([128, 2 * NB], F32)
                        nc.scalar.activation(
                            et[:], ps[:], Exp, accum_out=rs[:, col : col + 1]
                        )
                        dst = efold16[:, sl] if m == MCH - 1 else efold[:, sl]
                        nc.vector.tensor_add(dst, efold[:, sl], et[:])
            fold(NP2 - 1)

            nc.sync.dma_start(rowsum_d.ap(), rs[:])
            nc.sync.dma_start(colsum_d.ap(), colsb[:])

            # pre_cos reductions: dot on VectorE (scalar_tensor_tensor+accum),
            # norms on ScalarE (Square activation + accum). Scheduled after
            # the matmul stream in program order; fills engine idle time.
            prepool = ctx.enter_context(tc.tile_pool(name="prein", bufs=2))
            scrpool = ctx.enter_context(tc.tile_pool(name="prescr", bufs=2))
            for m in range(MCH):
                pv = prepool.tile([128, D], BF16, tag="pv")
                pa = prepool.tile([128, D], BF16, tag="pa")
                nc.gpsimd.dma_start(pv[:], preV_d.ap()[m])
                nc.gpsimd.dma_start(pa[:], preA_d.ap()[m])
                s = scrpool.tile([128, D], BF16, tag="scr")
                nc.vector.scalar_tensor_tensor(
                    out=s[:], in0=pv[:], scalar=1.0, in1=pa[:],
                    op0=mult, op1=mult,
                    accum_out=pre3[:, 3 * m : 3 * m + 1],
                )
                for j, a in ((1, pv), (2, pa)):
                    s2 = scrpool.tile([128, D], BF16, tag="scr2")
                    nc.scalar.activation(
                        s2[:], a[:], Square,
                        accum_out=pre3[:, 3 * m + j : 3 * m + j + 1],
                    )
            nc.sync.dma_start(pre3_d.ap(), pre3[:])

    nc.compile()
    return nc


def _get_nc():
    if "nc" not in _CACHE:
        _CACHE["nc"] = _build_nc()
    return _CACHE["nc"]


def _prep_inputs(pre_VF, pre_AF, back_VF, back_AF):
    """Normalize + relayout on host; returns per-core in_maps and host diag."""
    import ml_dtypes

    bf16 = ml_dtypes.bfloat16

    V = np.asarray(back_VF, dtype=np.float64)
    A = np.asarray(back_AF, dtype=np.float64)
    Vn = V / np.sqrt((V * V).sum(-1, keepdims=True) + EPS)
    An = A / np.sqrt((A * A).sum(-1, keepdims=True) + EPS)
    diag = np.einsum("ij,ij->i", Vn, An)  # f64, exact-ish

    Vn16 = Vn.astype(bf16)
    An16 = An.astype(bf16)

    # anT[n, p, k*NB + c] = An[n*NB + c, k*128 + p]
    anT = np.ascontiguousarray(
        An16.reshape(NCH, NB, KCH, 128).transpose(0, 3, 2, 1).reshape(NCH, 128, KCH * NB)
    )

    preV16 = np.asarray(pre_VF, dtype=np.float32).astype(bf16)
    preA16 = np.asarray(pre_AF, dtype=np.float32).astype(bf16)

    in_maps = []
    for c in range(NCORES):
        sl = slice(c * ROWS, (c + 1) * ROWS)
        # vnT[p, k*ROWS + m] = Vn_slab[m, k*128 + p]
        vnT = np.ascontiguousarray(
            Vn16[sl].reshape(ROWS, KCH, 128).transpose(2, 1, 0).reshape(128, KCH * ROWS)
        )
        in_maps.append(
            {
                "vnT": vnT,
                "anT": anT,
                "preV": np.ascontiguousarray(preV16[sl].reshape(MCH, 128, D)),
                "preA": np.ascontiguousarray(preA16[sl].reshape(MCH, 128, D)),
            }
        )
    return in_maps, diag


def _assemble(outs, diag):
    """O(N) final reduction on host, f64."""
    rowsum = np.concatenate(
        [
            outs[c]["rowsum"].astype(np.float64).reshape(128, NCH // 2, MCH).sum(1).T.reshape(ROWS)
            for c in range(NCORES)
        ]
    )
    colsum = np.zeros(N, dtype=np.float64)
    for c in range(NCORES):
        colsum += outs[c]["colsum"].astype(np.float64).reshape(N)
    pre = np.concatenate(
        [
            outs[c]["pre3"].astype(np.float64).reshape(128, MCH, 3).transpose(1, 0, 2).reshape(ROWS, 3)
            for c in range(NCORES)
        ]
    )
    dot, nv, na = pre[:, 0], pre[:, 1], pre[:, 2]

    dE = np.exp(diag)
    pos = np.exp(diag - MARGIN)
    neg_V = rowsum - dE
    neg_A = colsum - dE
    L_V = np.log(pos / (pos + neg_V)).sum()
    L_A = np.log(pos / (pos + neg_A)).sum()
    pre_cos = dot / (np.sqrt(nv + EPS) * np.sqrt(na + EPS))
    L_pre = pre_cos.sum()

    loss = BALANCE * (-1.0 / BIAS) * (L_V + L_A) + (1.0 - BALANCE) * L_pre
    return np.array(loss, dtype=np.float32)


def kernel(pre_VF, pre_AF, back_VF, back_AF):
    global LAST_RESULT
    from concourse import bass_utils

    nc = _get_nc()
    in_maps, diag = _prep_inputs(pre_VF, pre_AF, back_VF, back_AF)
    res = bass_utils.run_bass_kernel_spmd(nc, in_maps, core_ids=list(range(NCORES)))
    LAST_RESULT = res
    return _assemble(res.results, diag)


# revision 24
# speedup vs baseline: 1.1500x; 1.1500x over previous
"""Trainium2 kernel for nn_ContrastiveLoss (N=4096, D=1024), SPMD over 8 NeuronCores.

Strategy (row-sharded similarity matrix):
  - Host: l2-normalize back_VF/back_AF (f64 -> bf16), pre-transpose into
    matmul-friendly blocked layouts, compute the diagonal sims (O(N*D)).
  - Each core c: computes its [512, 4096] slab of E = exp(Vn @ An^T):
      * TensorE: 256 accumulating matmuls (K=1024 in 8 chunks, N in 8x512 cols)
      * ScalarE: exp(PSUM) -> bf16 SBUF tile, fused row-sum via accum_out
      * VectorE: partition-folded column-sum accumulation + pre_cos reductions
      * TensorE (ones-matmul): fold 128 partitions of the column sums
    Outputs per core: rowsum chunks [128, 32], partial colsum [1, 4096],
    pre-feature dot/normsq reductions [128, 12].
  - Host: O(N) final assembly (log/ratio/sums) in f64.
"""

import os
import sys

import numpy as np

for _p in ("/opt/trn_rl_repo",):
    if _p not in sys.path and os.path.isdir(_p):
        sys.path.insert(0, _p)

N = 4096
D = 1024
NCORES = 8
ROWS = N // NCORES       # 512 rows per core
MCH = ROWS // 128        # 4 row chunks per core
KCH = D // 128           # 8 contraction chunks
NB = 512                 # matmul moving free dim
NCH = N // NB            # 8 column blocks

MARGIN = 0.2
BALANCE = 0.5
BIAS = 1.0
EPS = 1e-18

KD2 = KCH // 2   # fp8 DoubleRow: contraction chunks of 256 (2 x 128 rows)
FP8_SCALE = 16.0  # host pre-scale so e4m3 keeps the values out of subnormals

_CACHE = {}
LAST_RESULT = None  # BassKernelResults of the most recent run (for test harness)


def _build_nc():
    import concourse.bass as bass  # noqa: F401
    import concourse.bacc as bacc
    import concourse.tile as tile
    from concourse import mybir
    from contextlib import ExitStack

    BF16 = mybir.dt.bfloat16
    F32 = mybir.dt.float32
    Exp = mybir.ActivationFunctionType.Exp
    Square = mybir.ActivationFunctionType.Square
    mult = mybir.AluOpType.mult

    NP2 = NCH // 2  # column-block pairs; each ACT/exp covers 1024 cols

    nc = bacc.Bacc("TRN2", debug=False, num_devices=NCORES)

    FP8 = mybir.dt.float8e4
    DoubleRow = mybir.MatmulPerfMode.DoubleRow

    # DRAM I/O (per core). Layouts chosen so every DMA is contiguous.
    # vnT[p, k2*2*ROWS + i*ROWS + m] = Vn_slab[m, (2*k2+i)*128 + p] * FP8_SCALE
    vnT_d = nc.dram_tensor("vnT", [128, KCH * ROWS], FP8, kind="ExternalInput")
    # anT[n, p, k2*2*NB + i*NB + c] = An[n*NB + c, (2*k2+i)*128 + p] * FP8_SCALE
    anT_d = nc.dram_tensor("anT", [NCH, 128, KCH * NB], FP8, kind="ExternalInput")
    # preX[m, p, :] = pre_X_slab[m*128 + p, :]
    preV_d = nc.dram_tensor("preV", [MCH, 128, D], BF16, kind="ExternalInput")
    preA_d = nc.dram_tensor("preA", [MCH, 128, D], BF16, kind="ExternalInput")

    # rowsum[p, np2*MCH + m] = sum over cols [np2*1024,(np2+1)*1024) of
    #   E_slab[m*128 + p, :]
    rowsum_d = nc.dram_tensor("rowsum", [128, NCH // 2 * MCH], F32, kind="ExternalOutput")
    # colsum[0, j] = sum over this core's 512 rows of E[:, j]
    colsum_d = nc.dram_tensor("colsum", [1, N], F32, kind="ExternalOutput")
    # pre3[p, 3*m + {0,1,2}] = dot/nv/na of slab row m*128+p
    pre3_d = nc.dram_tensor("pre3", [128, 3 * MCH], F32, kind="ExternalOutput")

    with tile.TileContext(nc) as tc:
        with ExitStack() as ctx:
            singles = ctx.enter_context(tc.tile_pool(name="singles", bufs=1))

            # DMA issue is ~600ns per dma_start on the issuing engine, so
            # spread issues over idle engines (round robin).
            dma_engines = [nc.sync, nc.scalar]
            _rr = [0]

            def dma(out_ap, in_ap):
                eng = dma_engines[_rr[0] % len(dma_engines)]
                _rr[0] += 1
                eng.dma_start(out_ap, in_ap)

            vn_sb = singles.tile([128, KCH * ROWS], FP8, tag="vn")
            half = KCH * ROWS // 2
            dma(vn_sb[:, :half], vnT_d.ap()[:, :half])
            dma(vn_sb[:, half:], vnT_d.ap()[:, half:])

            an_sb = []
            for n in range(NCH):
                t = singles.tile([128, KCH * NB], FP8, tag=f"an{n}")
                # the first blocks gate the matmul stream: split them wider
                # so more DMA queues work on them in parallel
                nsub = 8 if n < 2 else 4
                q = KCH * NB // nsub
                for j in range(nsub):
                    dma(t[:, j * q : (j + 1) * q], anT_d.ap()[n][:, j * q : (j + 1) * q])
                an_sb.append(t)

            efold = singles.tile([128, N], F32, tag="efold")
            efold16 = singles.tile([128, N], BF16, tag="efold16")
            rs = singles.tile([128, NP2 * MCH], F32, tag="rs")
            pre3 = singles.tile([128, 3 * MCH], F32, tag="pre3")
            ones_b = singles.tile([128, 1], BF16, tag="ones_b")
            nc.vector.memset(ones_b[:], 1.0)
            colsb = singles.tile([1, N], F32, tag="colsb")
            dummy = singles.tile([128, NB], BF16, tag="dummy")
            nc.vector.memset(dummy[:], 0.0)

            psum = ctx.enter_context(tc.tile_pool(name="mm_psum", bufs=3, space="PSUM"))
            foldp = ctx.enter_context(tc.tile_pool(name="fold_psum", bufs=2, space="PSUM"))
            epool = ctx.enter_context(tc.tile_pool(name="etile", bufs=3))

            # HAM warmup: keep TensorE busy during the initial DMA wait so the
            # clock gate is at 8/8 when the real matmul stream starts.
            wps = foldp.tile([128, NB], mybir.dt.float32, tag="fold")
            for i in range(8):
                nc.tensor.matmul(
                    wps[0:1, :], ones_b[:], dummy[:], start=(i == 0), stop=(i == 7)
                )

            # Main similarity slab. Column-pair outer (np2), row-chunk inner:
            # each group accumulates 16 matmuls into a [128, 1024] PSUM tile
            # (2 banks), then one wide exp (fused row-sum) drains it.
            # Column sums accumulate in f32 (m=0 written by exp directly,
            # m=3 add emits bf16) and are partition-folded by bf16
            # ones-matmuls, software-pipelined one pair behind the stream.
            def fold(np2):
                for j in range(2):
                    nn = 2 * np2 + j
                    fps = foldp.tile([128, NB], mybir.dt.float32, tag="fold")
                    nc.tensor.matmul(
                        fps[0:1, :],
                        ones_b[:],
                        efold16[:, nn * NB : (nn + 1) * NB],
                        start=True,
                        stop=True,
                    )
                    nc.scalar.copy(colsb[:, nn * NB : (nn + 1) * NB], fps[0:1, :])

            for np2 in range(NP2):
                nlo, nhi = 2 * np2, 2 * np2 + 1
                for m in range(MCH):
                    ps = psum.tile([128, 2 * NB], mybir.dt.float32)
                    for k2 in range(KD2):
                        w3 = (
                            vn_sb[:, k2 * 2 * ROWS : (k2 + 1) * 2 * ROWS]
                            .rearrange("p (i m) -> p i m", i=2)[
                                :, :, m * 128 : (m + 1) * 128
                            ]
                        )
                        for half, nn in ((0, nlo), (1, nhi)):
                            a3 = (
                                an_sb[nn][:, k2 * 2 * NB : (k2 + 1) * 2 * NB]
                                .rearrange("p (i c) -> p i c", i=2)
                            )
                            nc.tensor.matmul(
                                ps[:, half * NB : (half + 1) * NB],
                                w3,
                                a3,
                                start=(k2 == 0),
                                stop=(k2 == KD2 - 1),
                                perf_mode=DoubleRow,
                            )
                    if m == 0 and np2 > 0:
                        # previous pair's partition fold, emitted here so the
                        # PE never waits on the exp/add chain
                        fold(np2 - 1)
                    col = np2 * MCH + m
                    sl = slice(np2 * 2 * NB, (np2 + 1) * 2 * NB)
                    descale = 1.0 / (FP8_SCALE * FP8_SCALE)
                    if m == 0:
                        nc.scalar.activation(
                            efold[:, sl], ps[:], Exp, scale=descale,
                            accum_out=rs[:, col : col + 1],
                        )
                    else:
                        et = epool.tile([128, 2 * NB], F32)
                        nc.scalar.activation(
                            et[:], ps[:], Exp, scale=descale,
                            accum_out=rs[:, col : col + 1],
                        )
                        dst = efold16[:, sl] if m == MCH - 1 else efold[:, sl]
                        nc.vector.tensor_add(dst, efold[:, sl], et[:])
            fold(NP2 - 1)

            nc.sync.dma_start(rowsum_d.ap(), rs[:])
            nc.sync.dma_start(colsum_d.ap(), colsb[:])

            # pre_cos reductions: dot on VectorE (scalar_tensor_tensor+accum),
            # norms on ScalarE (Square activation + accum). Scheduled after
            # the matmul stream in program order; fills engine idle time.
            prepool = ctx.enter_context(tc.tile_pool(name="prein", bufs=2))
            scrpool = ctx.enter_context(tc.tile_pool(name="prescr", bufs=2))
            for m in range(MCH):
                pv = prepool.tile([128, D], BF16, tag="pv")
                pa = prepool.tile([128, D], BF16, tag="pa")
                nc.sync.dma_start(pv[:], preV_d.ap()[m])
                nc.sync.dma_start(pa[:], preA_d.ap()[m])
                s = scrpool.tile([128, D], BF16, tag="scr")
                nc.vector.scalar_tensor_tensor(
                    out=s[:], in0=pv[:], scalar=1.0, in1=pa[:],
                    op0=mult, op1=mult,
                    accum_out=pre3[:, 3 * m : 3 * m + 1],
                )
                for j, a in ((1, pv), (2, pa)):
                    s2 = scrpool.tile([128, D], BF16, tag="scr2")
                    nc.scalar.activation(
                        s2[:], a[:], Square,
                        accum_out=pre3[:, 3 * m + j : 3 * m + j + 1],
                    )
            nc.sync.dma_start(pre3_d.ap(), pre3[:])

    nc.compile()
    return nc


def _get_nc():
    if "nc" not in _CACHE:
        _CACHE["nc"] = _build_nc()
    return _CACHE["nc"]


def _prep_inputs(pre_VF, pre_AF, back_VF, back_AF):
    """Normalize + relayout on host; returns per-core in_maps and host diag."""
    import ml_dtypes

    bf16 = ml_dtypes.bfloat16

    V = np.asarray(back_VF, dtype=np.float64)
    A = np.asarray(back_AF, dtype=np.float64)
    Vn = V / np.sqrt((V * V).sum(-1, keepdims=True) + EPS)
    An = A / np.sqrt((A * A).sum(-1, keepdims=True) + EPS)
    diag = np.einsum("ij,ij->i", Vn, An)  # f64, exact-ish

    fp8 = ml_dtypes.float8_e4m3
    Vn8 = (Vn * FP8_SCALE).astype(fp8)
    An8 = (An * FP8_SCALE).astype(fp8)

    # anT[n, p, k2*2*NB + i*NB + c] = An8[n*NB + c, (2*k2+i)*128 + p]
    anT = np.ascontiguousarray(
        An8.reshape(NCH, NB, KD2, 2, 128)
        .transpose(0, 4, 2, 3, 1)
        .reshape(NCH, 128, KCH * NB)
    )

    preV16 = np.asarray(pre_VF, dtype=np.float32).astype(bf16)
    preA16 = np.asarray(pre_AF, dtype=np.float32).astype(bf16)

    in_maps = []
    for c in range(NCORES):
        sl = slice(c * ROWS, (c + 1) * ROWS)
        # vnT[p, k2*2*ROWS + i*ROWS + m] = Vn8_slab[m, (2*k2+i)*128 + p]
        vnT = np.ascontiguousarray(
            Vn8[sl]
            .reshape(ROWS, KD2, 2, 128)
            .transpose(3, 1, 2, 0)
            .reshape(128, KCH * ROWS)
        )
        in_maps.append(
            {
                "vnT": vnT,
                "anT": anT,
                "preV": np.ascontiguousarray(preV16[sl].reshape(MCH, 128, D)),
                "preA": np.ascontiguousarray(preA16[sl].reshape(MCH, 128, D)),
            }
        )
    return in_maps, diag


def _assemble(outs, diag):
    """O(N) final reduction on host, f64."""
    rowsum = np.concatenate(
        [
            outs[c]["rowsum"].astype(np.float64).reshape(128, NCH // 2, MCH).sum(1).T.reshape(ROWS)
            for c in range(NCORES)
        ]
    )
    colsum = np.zeros(N, dtype=np.float64)
    for c in range(NCORES):
        colsum += outs[c]["colsum"].astype(np.float64).reshape(N)
    pre = np.concatenate(
        [
            outs[c]["pre3"].astype(np.float64).reshape(128, MCH, 3).transpose(1, 0, 2).reshape(ROWS, 3)
            for c in range(NCORES)
        ]
    )
    dot, nv, na = pre[:, 0], pre[:, 1], pre[:, 2]

    dE = np.exp(diag)
    pos = np.exp(diag - MARGIN)
    neg_V = rowsum - dE
    neg_A = colsum - dE
    L_V = np.log(pos / (pos + neg_V)).sum()
    L_A = np.log(pos / (pos + neg_A)).sum()
    pre_cos = dot / (np.sqrt(nv + EPS) * np.sqrt(na + EPS))
    L_pre = pre_cos.sum()

    loss = BALANCE * (-1.0 / BIAS) * (L_V + L_A) + (1.0 - BALANCE) * L_pre
    return np.array(loss, dtype=np.float32)


def kernel(pre_VF, pre_AF, back_VF, back_AF):
    global LAST_RESULT
    from concourse import bass_utils

    nc = _get_nc()
    in_maps, diag = _prep_inputs(pre_VF, pre_AF, back_VF, back_AF)
    res = bass_utils.run_bass_kernel_spmd(nc, in_maps, core_ids=list(range(NCORES)))
    LAST_RESULT = res
    return _assemble(res.results, diag)


# revision 29
# speedup vs baseline: 1.2850x; 1.1174x over previous
"""Trainium2 kernel for nn_ContrastiveLoss (N=4096, D=1024), SPMD over 8 NeuronCores.

Strategy (row-sharded similarity matrix):
  - Host: l2-normalize back_VF/back_AF (f64 -> bf16), pre-transpose into
    matmul-friendly blocked layouts, compute the diagonal sims (O(N*D)).
  - Each core c: computes its [512, 4096] slab of E = exp(Vn @ An^T):
      * TensorE: 256 accumulating matmuls (K=1024 in 8 chunks, N in 8x512 cols)
      * ScalarE: exp(PSUM) -> bf16 SBUF tile, fused row-sum via accum_out
      * VectorE: partition-folded column-sum accumulation + pre_cos reductions
      * TensorE (ones-matmul): fold 128 partitions of the column sums
    Outputs per core: rowsum chunks [128, 32], partial colsum [1, 4096],
    pre-feature dot/normsq reductions [128, 12].
  - Host: O(N) final assembly (log/ratio/sums) in f64.
"""

import os
import sys

import numpy as np

for _p in ("/opt/trn_rl_repo",):
    if _p not in sys.path and os.path.isdir(_p):
        sys.path.insert(0, _p)

N = 4096
D = 1024
NCORES = 8
ROWS = N // NCORES       # 512 rows per core
MCH = ROWS // 128        # 4 row chunks per core
KCH = D // 128           # 8 contraction chunks
NB = 512                 # matmul moving free dim
NCH = N // NB            # 8 column blocks

MARGIN = 0.2
BALANCE = 0.5
BIAS = 1.0
EPS = 1e-18

KD2 = KCH // 2   # fp8 DoubleRow: contraction chunks of 256 (2 x 128 rows)
FP8_SCALE = 16.0  # host pre-scale so e4m3 keeps the values out of subnormals

_CACHE = {}
LAST_RESULT = None  # BassKernelResults of the most recent run (for test harness)


def _build_nc():
    import concourse.bass as bass  # noqa: F401
    import concourse.bacc as bacc
    import concourse.tile as tile
    from concourse import mybir
    from contextlib import ExitStack

    BF16 = mybir.dt.bfloat16
    F32 = mybir.dt.float32
    Exp = mybir.ActivationFunctionType.Exp
    Square = mybir.ActivationFunctionType.Square
    mult = mybir.AluOpType.mult

    NP2 = NCH // 2  # column-block pairs; each ACT/exp covers 1024 cols

    nc = bacc.Bacc("TRN2", debug=False, num_devices=NCORES)

    FP8 = mybir.dt.float8e4
    DoubleRow = mybir.MatmulPerfMode.DoubleRow

    # DRAM I/O (per core). Layouts chosen so every DMA is contiguous.
    # vnT[p, k2*2*ROWS + i*ROWS + m] = Vn_slab[m, (2*k2+i)*128 + p] * FP8_SCALE
    vnT_d = nc.dram_tensor("vnT", [128, KCH * ROWS], FP8, kind="ExternalInput")
    # anT[n, p, k2*2*NB + i*NB + c] = An[n*NB + c, (2*k2+i)*128 + p] * FP8_SCALE
    anT_d = nc.dram_tensor("anT", [NCH, 128, KCH * NB], FP8, kind="ExternalInput")
    # preX[m, p, :] = pre_X_slab[m*128 + p, :]
    preV_d = nc.dram_tensor("preV", [MCH, 128, D], BF16, kind="ExternalInput")
    preA_d = nc.dram_tensor("preA", [MCH, 128, D], BF16, kind="ExternalInput")

    # rowsum[p, np2*MCH + m] = sum over cols [np2*1024,(np2+1)*1024) of
    #   E_slab[m*128 + p, :]
    rowsum_d = nc.dram_tensor("rowsum", [128, NCH // 2 * MCH], F32, kind="ExternalOutput")
    # colsum[0, j] = sum over this core's 512 rows of E[:, j]
    colsum_d = nc.dram_tensor("colsum", [1, N], F32, kind="ExternalOutput")
    # pre3[p, 3*m + {0,1,2}] = dot/nv/na of slab row m*128+p
    pre3_d = nc.dram_tensor("pre3", [128, 3 * MCH], F32, kind="ExternalOutput")

    with tile.TileContext(nc) as tc:
        with ExitStack() as ctx:
            singles = ctx.enter_context(tc.tile_pool(name="singles", bufs=1))

            # DMA issue costs ~700ns on the issuing engine and one dma_start
            # lands on one ~11GB/s queue, so: split blocks into sub-DMAs for
            # queue parallelism, and spread the issue load by deadline --
            # early blocks on the two HWDGE engines (sync+scalar), the last
            # blocks on gpsimd's slower SWDGE queues (far deadline).
            def split_dma(engines, dst, src, nsub):
                q = dst.shape[-1] // nsub
                for j in range(nsub):
                    engines[j % len(engines)].dma_start(
                        dst[:, j * q : (j + 1) * q], src[:, j * q : (j + 1) * q]
                    )

            vn_sb = singles.tile([128, KCH * ROWS], FP8, tag="vn")
            split_dma([nc.scalar, nc.sync], vn_sb[:], vnT_d.ap(), 4)

            an_sb = []
            for n in range(NCH):
                an_t = singles.tile([128, KCH * NB], FP8, tag=f"an{n}")
                an_sb.append(an_t)
            for n in (0, 1):
                split_dma([nc.sync, nc.scalar], an_sb[n][:], anT_d.ap()[n], 8)

            # pre features: 2MB with a mid-kernel deadline -> scalar queue
            prepool = ctx.enter_context(tc.tile_pool(name="prein", bufs=4))
            pre_tiles = []
            for m in range(MCH):
                pv = prepool.tile([128, D], BF16, tag=f"pv{m}")
                pa = prepool.tile([128, D], BF16, tag=f"pa{m}")
                nc.scalar.dma_start(pv[:], preV_d.ap()[m])
                nc.scalar.dma_start(pa[:], preA_d.ap()[m])
                pre_tiles.append((pv, pa))

            for n in (2, 3, 4, 5):
                split_dma([nc.sync], an_sb[n][:], anT_d.ap()[n], 8)
            for n in (6, 7):
                split_dma([nc.gpsimd], an_sb[n][:], anT_d.ap()[n], 4)

            efold = singles.tile([128, N], F32, tag="efold")
            efold16 = singles.tile([128, N], BF16, tag="efold16")
            rs = singles.tile([128, NP2 * MCH], F32, tag="rs")
            pre3 = singles.tile([128, 3 * MCH], F32, tag="pre3")
            ones_b = singles.tile([128, 1], BF16, tag="ones_b")
            nc.vector.memset(ones_b[:], 1.0)
            colsb = singles.tile([1, N], F32, tag="colsb")
            dummy = singles.tile([128, NB], BF16, tag="dummy")
            nc.vector.memset(dummy[:], 0.0)

            psum = ctx.enter_context(tc.tile_pool(name="mm_psum", bufs=3, space="PSUM"))
            foldp = ctx.enter_context(tc.tile_pool(name="fold_psum", bufs=2, space="PSUM"))
            epool = ctx.enter_context(tc.tile_pool(name="etile", bufs=3))

            # HAM warmup: keep TensorE busy during the initial DMA wait so the
            # clock gate is at 8/8 when the real matmul stream starts.
            wps = foldp.tile([128, NB], mybir.dt.float32, tag="fold")
            for i in range(8):
                nc.tensor.matmul(
                    wps[0:1, :], ones_b[:], dummy[:], start=(i == 0), stop=(i == 7)
                )

            # pre_cos reductions: dot on VectorE (scalar_tensor_tensor+accum),
            # norms on ScalarE (Square activation + accum). Emitted before the
            # matmul stream so they fill engine idle time mid-kernel instead
            # of extending the tail.
            scrpool = ctx.enter_context(tc.tile_pool(name="prescr", bufs=2))
            for m in range(MCH):
                pv, pa = pre_tiles[m]
                s = scrpool.tile([128, D], BF16, tag="scr")
                nc.vector.scalar_tensor_tensor(
                    out=s[:], in0=pv[:], scalar=1.0, in1=pa[:],
                    op0=mult, op1=mult,
                    accum_out=pre3[:, 3 * m : 3 * m + 1],
                )
                for j, a in ((1, pv), (2, pa)):
                    s2 = scrpool.tile([128, D], BF16, tag="scr2")
                    nc.scalar.activation(
                        s2[:], a[:], Square,
                        accum_out=pre3[:, 3 * m + j : 3 * m + j + 1],
                    )
            nc.gpsimd.dma_start(pre3_d.ap(), pre3[:])

            # Main similarity slab. Column-pair outer (np2), row-chunk inner:
            # each group accumulates 16 matmuls into a [128, 1024] PSUM tile
            # (2 banks), then one wide exp (fused row-sum) drains it.
            # Column sums accumulate in f32 (m=0 written by exp directly,
            # m=3 add emits bf16) and are partition-folded by bf16
            # ones-matmuls, software-pipelined one pair behind the stream.
            def fold(np2):
                for j in range(2):
                    nn = 2 * np2 + j
                    fps = foldp.tile([128, NB], mybir.dt.float32, tag="fold")
                    nc.tensor.matmul(
                        fps[0:1, :],
                        ones_b[:],
                        efold16[:, nn * NB : (nn + 1) * NB],
                        start=True,
                        stop=True,
                    )
                    nc.scalar.copy(colsb[:, nn * NB : (nn + 1) * NB], fps[0:1, :])

            for np2 in range(NP2):
                nlo, nhi = 2 * np2, 2 * np2 + 1
                for m in range(MCH):
                    ps = psum.tile([128, 2 * NB], mybir.dt.float32)
                    for k2 in range(KD2):
                        w3 = (
                            vn_sb[:, k2 * 2 * ROWS : (k2 + 1) * 2 * ROWS]
                            .rearrange("p (i m) -> p i m", i=2)[
                                :, :, m * 128 : (m + 1) * 128
                            ]
                        )
                        for half, nn in ((0, nlo), (1, nhi)):
                            a3 = (
                                an_sb[nn][:, k2 * 2 * NB : (k2 + 1) * 2 * NB]
                                .rearrange("p (i c) -> p i c", i=2)
                            )
                            nc.tensor.matmul(
                                ps[:, half * NB : (half + 1) * NB],
                                w3,
                                a3,
                                start=(k2 == 0),
                                stop=(k2 == KD2 - 1),
                                perf_mode=DoubleRow,
                            )
                    if m == 0 and np2 > 0:
                        # previous pair's partition fold, emitted here so the
                        # PE never waits on the exp/add chain
                        fold(np2 - 1)
                    col = np2 * MCH + m
                    sl = slice(np2 * 2 * NB, (np2 + 1) * 2 * NB)
                    descale = 1.0 / (FP8_SCALE * FP8_SCALE)
                    if m == 0:
                        nc.scalar.activation(
                            efold[:, sl], ps[:], Exp, scale=descale,
                            accum_out=rs[:, col : col + 1],
                        )
                    else:
                        et = epool.tile([128, 2 * NB], F32)
                        nc.scalar.activation(
                            et[:], ps[:], Exp, scale=descale,
                            accum_out=rs[:, col : col + 1],
                        )
                        dst = efold16[:, sl] if m == MCH - 1 else efold[:, sl]
                        nc.vector.tensor_add(dst, efold[:, sl], et[:])
            fold(NP2 - 1)

            nc.sync.dma_start(rowsum_d.ap(), rs[:])
            nc.sync.dma_start(colsum_d.ap(), colsb[:])

    nc.compile()
    return nc


def _get_nc():
    if "nc" not in _CACHE:
        _CACHE["nc"] = _build_nc()
    return _CACHE["nc"]


def _prep_inputs(pre_VF, pre_AF, back_VF, back_AF):
    """Normalize + relayout on host; returns per-core in_maps and host diag."""
    import ml_dtypes

    bf16 = ml_dtypes.bfloat16

    V = np.asarray(back_VF, dtype=np.float64)
    A = np.asarray(back_AF, dtype=np.float64)
    Vn = V / np.sqrt((V * V).sum(-1, keepdims=True) + EPS)
    An = A / np.sqrt((A * A).sum(-1, keepdims=True) + EPS)
    diag = np.einsum("ij,ij->i", Vn, An)  # f64, exact-ish

    fp8 = ml_dtypes.float8_e4m3
    Vn8 = (Vn * FP8_SCALE).astype(fp8)
    An8 = (An * FP8_SCALE).astype(fp8)

    # anT[n, p, k2*2*NB + i*NB + c] = An8[n*NB + c, (2*k2+i)*128 + p]
    anT = np.ascontiguousarray(
        An8.reshape(NCH, NB, KD2, 2, 128)
        .transpose(0, 4, 2, 3, 1)
        .reshape(NCH, 128, KCH * NB)
    )

    preV16 = np.asarray(pre_VF, dtype=np.float32).astype(bf16)
    preA16 = np.asarray(pre_AF, dtype=np.float32).astype(bf16)

    in_maps = []
    for c in range(NCORES):
        sl = slice(c * ROWS, (c + 1) * ROWS)
        # vnT[p, k2*2*ROWS + i*ROWS + m] = Vn8_slab[m, (2*k2+i)*128 + p]
        vnT = np.ascontiguousarray(
            Vn8[sl]
            .reshape(ROWS, KD2, 2, 128)
            .transpose(3, 1, 2, 0)
            .reshape(128, KCH * ROWS)
        )
        in_maps.append(
            {
                "vnT": vnT,
                "anT": anT,
                "preV": np.ascontiguousarray(preV16[sl].reshape(MCH, 128, D)),
                "preA": np.ascontiguousarray(preA16[sl].reshape(MCH, 128, D)),
            }
        )
    return in_maps, diag


def _assemble(outs, diag):
    """O(N) final reduction on host, f64."""
    rowsum = np.concatenate(
        [
            outs[c]["rowsum"].astype(np.float64).reshape(128, NCH // 2, MCH).sum(1).T.reshape(ROWS)
            for c in range(NCORES)
        ]
    )
    colsum = np.zeros(N, dtype=np.float64)
    for c in range(NCORES):
        colsum += outs[c]["colsum"].astype(np.float64).reshape(N)
    pre = np.concatenate(
        [
            outs[c]["pre3"].astype(np.float64).reshape(128, MCH, 3).transpose(1, 0, 2).reshape(ROWS, 3)
            for c in range(NCORES)
        ]
    )
    dot, nv, na = pre[:, 0], pre[:, 1], pre[:, 2]

    dE = np.exp(diag)
    pos = np.exp(diag - MARGIN)
    neg_V = rowsum - dE
    neg_A = colsum - dE
    L_V = np.log(pos / (pos + neg_V)).sum()
    L_A = np.log(pos / (pos + neg_A)).sum()
    pre_cos = dot / (np.sqrt(nv + EPS) * np.sqrt(na + EPS))
    L_pre = pre_cos.sum()

    loss = BALANCE * (-1.0 / BIAS) * (L_V + L_A) + (1.0 - BALANCE) * L_pre
    return np.array(loss, dtype=np.float32)


def kernel(pre_VF, pre_AF, back_VF, back_AF):
    global LAST_RESULT
    from concourse import bass_utils

    nc = _get_nc()
    in_maps, diag = _prep_inputs(pre_VF, pre_AF, back_VF, back_AF)
    res = bass_utils.run_bass_kernel_spmd(nc, in_maps, core_ids=list(range(NCORES)))
    LAST_RESULT = res
    return _assemble(res.results, diag)
